# revision 61
# baseline (speedup 1.0000x reference)
"""Trainium2 Bass kernel for the CrossEntropyMap loss.

Math (per batch row b of y_hat[B=64, T=64, G=128, G]):
    lse_b  = logsumexp(y_hat[b].reshape(-1))            # over T*G*G = 1M classes
    pick_b = sum_t y_hat[b, t, xi[b,t], yi[b,t]]        # xi/yi = round(coords*G)
    loss   = mean_b(T * lse_b - pick_b)

Sharding: data-parallel over batch, 8 rows per NeuronCore. The host converts
each core's shard to fp8-e4m3 before upload: the per-core HBM roofline
(~360 GB/s) is the binding constraint and the 2e-2 loss tolerance leaves
~100x headroom for the ~1e-4 relative effect fp8 rounding has on lse_b
(x ~ N(0,1); quantization noise is symmetric and averages out across the
1M-class sum). The picked logits are gathered on the host from the same
fp8 shard it uploads (bit-identical to a device-side gather; the indices
are host-computed either way) so no indirect-DMA machinery is needed.

The 54.6us serial ACT bottleneck of a pure exp+accum kernel (0.833ns per
free-axis element, dtype-independent) is split three ways per row tile:
  - ACT: exact exp(x + C_SHIFT) + accum on each chunk's leading columns
  - DVE: Schraudolph fast-exp on the rest: one 2x-mode tensor_scalar
    u8 = round(x*A5 + B5) emits the fp8-e5m2 BIT PATTERN of e^x
    (linear-mantissa approx, ~13% sawtooth whose mean is calibrated out
    by BETA5)
  - PE: sums the e5m2 codes with dual-row fp8 matmuls against ones
    (1024 columns per 512-cycle-pair instruction; remainders use plain
    fp8 matmuls), accumulating row r in PSUM bank r; the all-ones
    [128, 2, 128] stationary replicates the row sum to all partitions
    (dual-fp8 LDWEIGHTS requires full column groups, and partition-
    offset PSUM outputs fail the s3d3 dst-partition ISA check, so
    quadrant-packing rows into fewer banks is not codegen-able)
  - DVE drains each bank's [1, 512] residue to a scalar (GpSimd cannot
    read PSUM); reduces are emitted two rows late so they never stall
    DVE behind PE's accumulation-stop latency
Per ROW_PLAN, early rows run as half-row tiles so compute starts while
the DMA engines ramp up, and the last row leans on ACT so the trailing
DVE->PE->reduce chain is short. A dummy activation warms the ACT Exp
table before data lands. GpSimd is left idle on purpose: offloading a
transform slice to it measured slower (software-DSP jitter up to 4x) and
the device's activity throttle caps aggregate engine utilization anyway.

The per-chunk per-partition ACT sums and the 8 row scalars are returned
raw in one [128, N_ACC+8] f32 block and combined on the host in f64:
    S_r = sum_p sact[p,r] + exp(C_SHIFT)*sd[r]/BETA5
    partial_c = T * sum_r (ln S_r - C_SHIFT) - sum picks_c
"""

import sys

import numpy as np

try:
    import concourse.bacc as bacc
except ImportError:  # pragma: no cover - fallback for bare environments
    sys.path.insert(0, "/opt/trn_rl_repo")
    import concourse.bacc as bacc

import concourse.tile as tile
from concourse import mybir
from concourse.bass_utils import run_bass_kernel_spmd

B, T, G = 64, 64, 128
N_CORES = 8
ROWS = B // N_CORES            # 8 batch rows per core
ROW_ELEMS = T * G * G          # 1_048_576 classes per row
P = 128
F = ROW_ELEMS // P             # 8192 elements per partition per row
N_PER_CORE = ROWS * ROW_ELEMS  # 8_388_608 elements per core shard
C_SHIFT = -16.0                # constant exp bias on the ACT path

# Per-row chunking: (chunks, ACT columns per chunk, GpSimd columns per
# chunk; DVE takes the rest). Early rows use small chunks so compute
# starts while the DMA engines are still ramping; row 7 leans on ACT so
# the trailing PE+reduce chain is short. Mid-stream rows hand a small
# slice of the fast-exp transform to the otherwise idle GpSimd — small
# enough that its software-DSP jitter (~2x) stays under the row pace;
# the ramp rows and the tail row stay off it. The ACT/DVE split
# (~0.8ns vs ~0.55ns per column) keeps both at the same chunk latency.
ROW_PLAN = [
    [(2048, 768), (6144, 2304)],   # row 0: quarter then 3/4 — compute
    [(4096, 1536), (4096, 1536)],  #   starts while DMA still ramps
    [(4096, 1536), (4096, 1536)],  # rows 1-2: half tiles
    [(8192, 3584)],                # rows 3-6: whole tiles; FA/FD equalizes
    [(8192, 3584)],                #   ACT vs DVE+bank-reduce load
    [(8192, 3584)],
    [(8192, 3584)],
    [(8192, 5120)],                # row 7
]
N_ACC = sum(len(cs) for cs in ROW_PLAN)

# Schraudolph constants: u8 = convert(x * A5 + B5) is the e5m2 bit pattern
# of ~e^x. A5 = 4*log2(e); B5 centers the linear-mantissa sawtooth (mean
# relative error ~zero under the N(0,1)-induced uniform mantissa-phase
# distribution). BETA5 is the residual calibration factor of the summed
# approximation, measured offline over 30M samples of the full pipeline
# (fp8-e4m3 input quantization -> f32 FMA -> u8 convert -> e5m2 decode).
# The hardware f32->u8 convert rounds to nearest (verified on-device:
# the truncation hypothesis was off by the predicted +3.2 in the loss,
# round-to-nearest lands within 6e-5), so B5 carries a -0.5 offset
# relative to the floor-semantics constant.
A5 = 5.770780163555853         # 4 * log2(e)
B5 = 59.774399
BETA5 = 0.99838459

IN_DTYPE = mybir.dt.float8e4   # ml_dtypes.float8_e4m3 on the host side

_f32 = mybir.dt.float32
_u8 = mybir.dt.uint8
_bf16 = mybir.dt.bfloat16
_fp8e5 = mybir.dt.float8e5
_EXP = mybir.ActivationFunctionType.Exp
_AXF = mybir.AxisListType.X
_ADD = mybir.AluOpType.add
_MUL = mybir.AluOpType.mult
_DROW = mybir.MatmulPerfMode.DoubleRow

_compiled_nc = None

# Test hook: BassKernelResults of the last run.
LAST_RESULTS = None


def build_nc():
    nc = bacc.Bacc("TRN2", target_bir_lowering=False, debug=False)
    y = nc.dram_tensor("y", [N_PER_CORE, 1], IN_DTYPE, kind="ExternalInput")
    # One [128, N_ACC+8] f32 result block: cols [0, N_ACC) = per-partition
    # ACT sums, one per chunk in ROW_PLAN order; the last 8 cols = fast-exp
    # row sums (valid at partition 0 only).
    out_d = nc.dram_tensor(
        "res", [P, N_ACC + ROWS], _f32, kind="ExternalOutput"
    )

    # Row view: partition p of row r holds elements [r*1M + p*8192,
    # +8192) — contiguous per partition (8 KiB fp8); chunks are column
    # slices of it.
    y_rows = y.ap().rearrange("(r p f) o -> r p (f o)", r=ROWS, p=P)

    with tile.TileContext(nc) as tc:
        with (
            tc.tile_pool(name="xpool", bufs=1) as xpool,
            tc.tile_pool(name="ea", bufs=2) as eapool,
            tc.tile_pool(name="ed", bufs=3) as edpool,
            tc.tile_pool(name="small", bufs=1) as small,
            tc.tile_pool(name="psum", bufs=1, space="PSUM") as psum,
        ):
            ones8 = small.tile([P, 256], _fp8e5)
            nc.vector.memset(ones8[:], 1.0)
            cbias = small.tile([P, 1], _f32)
            nc.vector.memset(cbias[:], C_SHIFT)
            combo = small.tile([P, N_ACC + ROWS], _f32)
            pd = [
                psum.tile([P, 512], _f32, tag=f"pd{b}", name=f"pd{b}")
                for b in range(ROWS)
            ]
            # Warm the ACT Exp table before row 0 lands.
            warm = small.tile([P, 1], _f32)
            nc.scalar.activation(out=warm[:], in_=cbias[:], func=_EXP)

            # Chunk loads per ROW_PLAN, all on the SP HWDGE ring; the DMA
            # engines serialize them in issue order at ~400 GB/s.
            x_tiles = {}
            for r, chunks in enumerate(ROW_PLAN):
                off = 0
                for h, (w, _) in enumerate(chunks):
                    xt = xpool.tile(
                        [P, w], IN_DTYPE, tag=f"x{r}_{h}", bufs=1,
                        name=f"x{r}_{h}",
                    )
                    nc.sync.dma_start(
                        out=xt[:], in_=y_rows[r][:, off : off + w]
                    )
                    x_tiles[(r, h)] = xt
                    off += w

            lhs = ones8[:].rearrange("p (two m) -> p two m", two=2)

            def dr_matmuls(bank, code_tile, ncols, first, last):
                """Sum `ncols` e5m2 codes into bank: 1024-wide dual-row
                matmuls plus plain-matmul remainders (512/256)."""
                e5 = code_tile[:].bitcast(_fp8e5)
                ops = []
                off = 0
                while off + 1024 <= ncols:
                    ops.append((off, 1024, True))
                    off += 1024
                while off < ncols:
                    w = 512 if off + 512 <= ncols else ncols - off
                    ops.append((off, w, False))
                    off += w
                for i, (o, w, dual) in enumerate(ops):
                    if dual:
                        rhs = e5[:, o : o + 1024].rearrange(
                            "p (two f) -> p two f", two=2
                        )
                        nc.tensor.matmul(
                            out=bank[:, :], lhsT=lhs, rhs=rhs,
                            start=(first and i == 0),
                            stop=(last and i == len(ops) - 1),
                            perf_mode=_DROW,
                        )
                    else:
                        nc.tensor.matmul(
                            out=bank[:, 0:w], lhsT=ones8[:, 0:P],
                            rhs=e5[:, o : o + w],
                            start=(first and i == 0),
                            stop=(last and i == len(ops) - 1),
                        )

            def bank_reduce(r):
                # Drain row r's bank to a scalar (GpSimd cannot read PSUM,
                # so this rides the DVE queue; emitted two rows late so it
                # never stalls DVE on PE's accumulation-stop latency).
                nc.vector.tensor_reduce(
                    out=combo[0:1, N_ACC + r : N_ACC + r + 1],
                    in_=pd[r][0:1, :], axis=_AXF, op=_ADD,
                )

            acc_col = 0
            for r, chunks in enumerate(ROW_PLAN):
                bank = pd[r]
                for h, (w, fa) in enumerate(chunks):
                    fd = w - fa
                    xt = x_tiles[(r, h)]
                    ea = eapool.tile(
                        [P, fa], _bf16, tag="ea", name=f"ea{r}_{h}"
                    )
                    nc.scalar.activation(
                        out=ea[:], in_=xt[:, 0:fa], func=_EXP,
                        bias=cbias[:, 0:1], scale=1.0,
                        accum_out=combo[:, acc_col : acc_col + 1],
                    )
                    acc_col += 1
                    ed = edpool.tile(
                        [P, fd], _u8, tag="ed", name=f"ed{r}_{h}"
                    )
                    nc.vector.tensor_scalar(
                        out=ed[:], in0=xt[:, fa:w],
                        scalar1=float(A5), scalar2=float(B5),
                        op0=_MUL, op1=_ADD,
                    )
                    dr_matmuls(
                        bank, ed, fd, first=(h == 0),
                        last=(h == len(chunks) - 1),
                    )
                if r >= 2:
                    bank_reduce(r - 2)
            bank_reduce(ROWS - 2)
            bank_reduce(ROWS - 1)

            nc.sync.dma_start(out=out_d.ap(), in_=combo[:])

    nc.compile()
    return nc


def make_in_maps(y_hat: np.ndarray, coords: np.ndarray):
    """Shard y_hat (as fp8) and host-gather the picked logits per core."""
    np_in_dtype = mybir.dt.np(IN_DTYPE)
    coords = np.asarray(coords, dtype=np.float32)

    # Match jnp.round (round-half-to-even); np.round has identical semantics,
    # and coords * 128 is exact in f32 (power-of-two scale).
    xi = np.round(coords[:, :, 0] * np.float32(G)).astype(np.int64)  # (B, T)
    yi = np.round(coords[:, :, 1] * np.float32(G)).astype(np.int64)  # (B, T)
    t = np.arange(T, dtype=np.int64)[None, :]
    flat = t * (G * G) + xi * G + yi  # (B, T) element offset within row b

    in_maps = []
    pick_sums = []
    for c in range(N_CORES):
        rows = slice(c * ROWS, (c + 1) * ROWS)
        shard = np.ascontiguousarray(y_hat[rows]).astype(np_in_dtype)
        shard = shard.reshape(N_PER_CORE, 1)
        local = np.arange(ROWS, dtype=np.int64)[:, None] * ROW_ELEMS + flat[rows]
        # Same fp8 values a device-side gather would read.
        pick_sums.append(
            shard[local.reshape(-1), 0].astype(np.float64).sum()
        )
        in_maps.append({"y": shard})
    return in_maps, pick_sums


def kernel(y_hat: np.ndarray, coords: np.ndarray) -> np.ndarray:
    global _compiled_nc, LAST_RESULTS
    in_maps, pick_sums = make_in_maps(y_hat, coords)
    if _compiled_nc is None:
        _compiled_nc = build_nc()
    try:
        res = run_bass_kernel_spmd(
            _compiled_nc, in_maps, core_ids=list(range(N_CORES))
        )
    except Exception:
        # One retry: a transient NRT_EXEC_UNIT_UNRECOVERABLE was observed
        # once across many runs; the immediate retry succeeded.
        res = run_bass_kernel_spmd(
            _compiled_nc, in_maps, core_ids=list(range(N_CORES))
        )
    LAST_RESULTS = res
    total = 0.0
    scale_d = np.exp(np.float64(C_SHIFT)) / BETA5
    for c, r in enumerate(res.results):
        blk = np.asarray(r["res"], dtype=np.float64)        # [P, N_ACC+ROWS]
        acc = blk[:, :N_ACC].sum(axis=0)                    # per accum col
        sact = np.empty(ROWS)
        col = 0
        for i, chunks in enumerate(ROW_PLAN):
            sact[i] = acc[col : col + len(chunks)].sum()
            col += len(chunks)
        sd = blk[0, N_ACC : N_ACC + ROWS]                   # [ROWS]
        s_tot = sact + scale_d * sd                         # [ROWS]
        lse = np.log(s_tot) - C_SHIFT
        total += T * lse.sum() - pick_sums[c]
    loss = total / B
    return np.array(np.float32(loss))


# revision 62
# speedup vs baseline: 1.0337x; 1.0337x over previous
"""Trainium2 Bass kernel for the CrossEntropyMap loss.

Math (per batch row b of y_hat[B=64, T=64, G=128, G]):
    lse_b  = logsumexp(y_hat[b].reshape(-1))            # over T*G*G = 1M classes
    pick_b = sum_t y_hat[b, t, xi[b,t], yi[b,t]]        # xi/yi = round(coords*G)
    loss   = mean_b(T * lse_b - pick_b)

Sharding: data-parallel over batch, 8 rows per NeuronCore. The host converts
each core's shard to fp8-e4m3 before upload: the per-core HBM roofline
(~360 GB/s) is the binding constraint and the 2e-2 loss tolerance leaves
~100x headroom for the ~1e-4 relative effect fp8 rounding has on lse_b
(x ~ N(0,1); quantization noise is symmetric and averages out across the
1M-class sum). The picked logits are gathered on the host from the same
fp8 shard it uploads (bit-identical to a device-side gather; the indices
are host-computed either way) so no indirect-DMA machinery is needed.

The 54.6us serial ACT bottleneck of a pure exp+accum kernel (0.833ns per
free-axis element, dtype-independent) is split three ways per row tile:
  - ACT: exact exp(x + C_SHIFT) + accum on each chunk's leading columns
  - DVE: Schraudolph fast-exp on the rest: one 2x-mode tensor_scalar
    u8 = round(x*A5 + B5) emits the fp8-e5m2 BIT PATTERN of e^x
    (linear-mantissa approx, ~13% sawtooth whose mean is calibrated out
    by BETA5)
  - PE: sums the e5m2 codes with dual-row fp8 matmuls against ones
    (1024 columns per 512-cycle-pair instruction; remainders use plain
    fp8 matmuls), accumulating row r in PSUM bank r; the all-ones
    [128, 2, 128] stationary replicates the row sum to all partitions
    (dual-fp8 LDWEIGHTS requires full column groups, and partition-
    offset PSUM outputs fail the s3d3 dst-partition ISA check, so
    quadrant-packing rows into fewer banks is not codegen-able)
  - DVE drains each bank's [1, 512] residue to a scalar (GpSimd cannot
    read PSUM); reduces are emitted two rows late so they never stall
    DVE behind PE's accumulation-stop latency
Per ROW_PLAN, early rows run as half-row tiles so compute starts while
the DMA engines ramp up, and the last row leans on ACT so the trailing
DVE->PE->reduce chain is short. A dummy activation warms the ACT Exp
table before data lands. GpSimd is left idle on purpose: offloading a
transform slice to it measured slower (software-DSP jitter up to 4x) and
the device's activity throttle caps aggregate engine utilization anyway.

The per-chunk per-partition ACT sums and the 8 row scalars are returned
raw in one [128, N_ACC+8] f32 block and combined on the host in f64:
    S_r = sum_p sact[p,r] + exp(C_SHIFT)*sd[r]/BETA5
    partial_c = T * sum_r (ln S_r - C_SHIFT) - sum picks_c
"""

import sys

import numpy as np

try:
    import concourse.bacc as bacc
except ImportError:  # pragma: no cover - fallback for bare environments
    sys.path.insert(0, "/opt/trn_rl_repo")
    import concourse.bacc as bacc

import concourse.tile as tile
from concourse import mybir
from concourse.bass_utils import run_bass_kernel_spmd

B, T, G = 64, 64, 128
N_CORES = 8
ROWS = B // N_CORES            # 8 batch rows per core
ROW_ELEMS = T * G * G          # 1_048_576 classes per row
P = 128
F = ROW_ELEMS // P             # 8192 elements per partition per row
N_PER_CORE = ROWS * ROW_ELEMS  # 8_388_608 elements per core shard
C_SHIFT = -16.0                # constant exp bias on the ACT path

# Per-row chunking: (chunks, ACT columns per chunk, GpSimd columns per
# chunk; DVE takes the rest). Early rows use small chunks so compute
# starts while the DMA engines are still ramping; row 7 leans on ACT so
# the trailing PE+reduce chain is short. Mid-stream rows hand a small
# slice of the fast-exp transform to the otherwise idle GpSimd — small
# enough that its software-DSP jitter (~2x) stays under the row pace;
# the ramp rows and the tail row stay off it. The ACT/DVE split
# (~0.8ns vs ~0.55ns per column) keeps both at the same chunk latency.
ROW_PLAN = [
    [(4096, 1536), (4096, 1536)],  # rows 0-2: half tiles — compute starts
    [(4096, 1536), (4096, 1536)],  #   while the DMA engines still ramp
    [(4096, 1536), (4096, 1536)],
    [(8192, 3072)],                # rows 3-6: whole tiles
    [(8192, 3072)],
    [(8192, 3072)],
    [(8192, 3072)],
    [(8192, 5120)],                # row 7: leans on ACT for a short tail
]
N_ACC = sum(len(cs) for cs in ROW_PLAN)

# Schraudolph constants: u8 = convert(x * A5 + B5) is the e5m2 bit pattern
# of ~e^x. A5 = 4*log2(e); B5 centers the linear-mantissa sawtooth (mean
# relative error ~zero under the N(0,1)-induced uniform mantissa-phase
# distribution). BETA5 is the residual calibration factor of the summed
# approximation, measured offline over 30M samples of the full pipeline
# (fp8-e4m3 input quantization -> f32 FMA -> u8 convert -> e5m2 decode).
# The hardware f32->u8 convert rounds to nearest (verified on-device:
# the truncation hypothesis was off by the predicted +3.2 in the loss,
# round-to-nearest lands within 6e-5), so B5 carries a -0.5 offset
# relative to the floor-semantics constant.
A5 = 5.770780163555853         # 4 * log2(e)
B5 = 59.774399
BETA5 = 0.99838459

IN_DTYPE = mybir.dt.float8e4   # ml_dtypes.float8_e4m3 on the host side

_f32 = mybir.dt.float32
_u8 = mybir.dt.uint8
_bf16 = mybir.dt.bfloat16
_fp8e5 = mybir.dt.float8e5
_EXP = mybir.ActivationFunctionType.Exp
_AXF = mybir.AxisListType.X
_ADD = mybir.AluOpType.add
_MUL = mybir.AluOpType.mult
_DROW = mybir.MatmulPerfMode.DoubleRow

_compiled_nc = None

# Test hook: BassKernelResults of the last run.
LAST_RESULTS = None


def build_nc():
    nc = bacc.Bacc("TRN2", target_bir_lowering=False, debug=False)
    y = nc.dram_tensor("y", [N_PER_CORE, 1], IN_DTYPE, kind="ExternalInput")
    # One [128, N_ACC+8] f32 result block: cols [0, N_ACC) = per-partition
    # ACT sums, one per chunk in ROW_PLAN order; the last 8 cols = fast-exp
    # row sums (valid at partition 0 only).
    out_d = nc.dram_tensor(
        "res", [P, N_ACC + ROWS], _f32, kind="ExternalOutput"
    )

    # Row view: partition p of row r holds elements [r*1M + p*8192,
    # +8192) — contiguous per partition (8 KiB fp8); chunks are column
    # slices of it.
    y_rows = y.ap().rearrange("(r p f) o -> r p (f o)", r=ROWS, p=P)

    with tile.TileContext(nc) as tc:
        with (
            tc.tile_pool(name="xpool", bufs=1) as xpool,
            tc.tile_pool(name="ea", bufs=2) as eapool,
            tc.tile_pool(name="ed", bufs=3) as edpool,
            tc.tile_pool(name="small", bufs=1) as small,
            tc.tile_pool(name="psum", bufs=1, space="PSUM") as psum,
        ):
            ones8 = small.tile([P, 256], _fp8e5)
            nc.vector.memset(ones8[:], 1.0)
            cbias = small.tile([P, 1], _f32)
            nc.vector.memset(cbias[:], C_SHIFT)
            combo = small.tile([P, N_ACC + ROWS], _f32)
            pd = [
                psum.tile([P, 512], _f32, tag=f"pd{b}", name=f"pd{b}")
                for b in range(ROWS)
            ]
            # Warm the ACT Exp table before row 0 lands.
            warm = small.tile([P, 1], _f32)
            nc.scalar.activation(out=warm[:], in_=cbias[:], func=_EXP)

            # Chunk loads per ROW_PLAN, all on the SP HWDGE ring; the DMA
            # engines serialize them in issue order at ~400 GB/s.
            x_tiles = {}
            for r, chunks in enumerate(ROW_PLAN):
                off = 0
                for h, (w, _) in enumerate(chunks):
                    xt = xpool.tile(
                        [P, w], IN_DTYPE, tag=f"x{r}_{h}", bufs=1,
                        name=f"x{r}_{h}",
                    )
                    nc.sync.dma_start(
                        out=xt[:], in_=y_rows[r][:, off : off + w]
                    )
                    x_tiles[(r, h)] = xt
                    off += w

            lhs = ones8[:].rearrange("p (two m) -> p two m", two=2)

            def dr_matmuls(bank, code_tile, ncols, first, last):
                """Sum `ncols` e5m2 codes into bank: 1024-wide dual-row
                matmuls plus plain-matmul remainders (512/256)."""
                e5 = code_tile[:].bitcast(_fp8e5)
                ops = []
                off = 0
                while off + 1024 <= ncols:
                    ops.append((off, 1024, True))
                    off += 1024
                while off < ncols:
                    w = 512 if off + 512 <= ncols else ncols - off
                    ops.append((off, w, False))
                    off += w
                for i, (o, w, dual) in enumerate(ops):
                    if dual:
                        rhs = e5[:, o : o + 1024].rearrange(
                            "p (two f) -> p two f", two=2
                        )
                        nc.tensor.matmul(
                            out=bank[:, :], lhsT=lhs, rhs=rhs,
                            start=(first and i == 0),
                            stop=(last and i == len(ops) - 1),
                            perf_mode=_DROW,
                        )
                    else:
                        nc.tensor.matmul(
                            out=bank[:, 0:w], lhsT=ones8[:, 0:P],
                            rhs=e5[:, o : o + w],
                            start=(first and i == 0),
                            stop=(last and i == len(ops) - 1),
                        )

            def bank_reduce(r):
                # Drain row r's bank to a scalar (GpSimd cannot read PSUM,
                # so this rides the DVE queue; emitted two rows late so it
                # never stalls DVE on PE's accumulation-stop latency).
                nc.vector.tensor_reduce(
                    out=combo[0:1, N_ACC + r : N_ACC + r + 1],
                    in_=pd[r][0:1, :], axis=_AXF, op=_ADD,
                )

            acc_col = 0
            for r, chunks in enumerate(ROW_PLAN):
                bank = pd[r]
                for h, (w, fa) in enumerate(chunks):
                    fd = w - fa
                    xt = x_tiles[(r, h)]
                    ea = eapool.tile(
                        [P, fa], _bf16, tag="ea", name=f"ea{r}_{h}"
                    )
                    nc.scalar.activation(
                        out=ea[:], in_=xt[:, 0:fa], func=_EXP,
                        bias=cbias[:, 0:1], scale=1.0,
                        accum_out=combo[:, acc_col : acc_col + 1],
                    )
                    acc_col += 1
                    ed = edpool.tile(
                        [P, fd], _u8, tag="ed", name=f"ed{r}_{h}"
                    )
                    nc.vector.tensor_scalar(
                        out=ed[:], in0=xt[:, fa:w],
                        scalar1=float(A5), scalar2=float(B5),
                        op0=_MUL, op1=_ADD,
                    )
                    dr_matmuls(
                        bank, ed, fd, first=(h == 0),
                        last=(h == len(chunks) - 1),
                    )
                if r >= 2:
                    bank_reduce(r - 2)
            bank_reduce(ROWS - 2)
            bank_reduce(ROWS - 1)

            nc.sync.dma_start(out=out_d.ap(), in_=combo[:])

    nc.compile()
    return nc


def make_in_maps(y_hat: np.ndarray, coords: np.ndarray):
    """Shard y_hat (as fp8) and host-gather the picked logits per core."""
    np_in_dtype = mybir.dt.np(IN_DTYPE)
    coords = np.asarray(coords, dtype=np.float32)

    # Match jnp.round (round-half-to-even); np.round has identical semantics,
    # and coords * 128 is exact in f32 (power-of-two scale).
    xi = np.round(coords[:, :, 0] * np.float32(G)).astype(np.int64)  # (B, T)
    yi = np.round(coords[:, :, 1] * np.float32(G)).astype(np.int64)  # (B, T)
    t = np.arange(T, dtype=np.int64)[None, :]
    flat = t * (G * G) + xi * G + yi  # (B, T) element offset within row b

    in_maps = []
    pick_sums = []
    for c in range(N_CORES):
        rows = slice(c * ROWS, (c + 1) * ROWS)
        shard = np.ascontiguousarray(y_hat[rows]).astype(np_in_dtype)
        shard = shard.reshape(N_PER_CORE, 1)
        local = np.arange(ROWS, dtype=np.int64)[:, None] * ROW_ELEMS + flat[rows]
        # Same fp8 values a device-side gather would read.
        pick_sums.append(
            shard[local.reshape(-1), 0].astype(np.float64).sum()
        )
        in_maps.append({"y": shard})
    return in_maps, pick_sums


def kernel(y_hat: np.ndarray, coords: np.ndarray) -> np.ndarray:
    global _compiled_nc, LAST_RESULTS
    in_maps, pick_sums = make_in_maps(y_hat, coords)
    if _compiled_nc is None:
        _compiled_nc = build_nc()
    try:
        res = run_bass_kernel_spmd(
            _compiled_nc, in_maps, core_ids=list(range(N_CORES))
        )
    except Exception:
        # One retry: a transient NRT_EXEC_UNIT_UNRECOVERABLE was observed
        # once across many runs; the immediate retry succeeded.
        res = run_bass_kernel_spmd(
            _compiled_nc, in_maps, core_ids=list(range(N_CORES))
        )
    LAST_RESULTS = res
    total = 0.0
    scale_d = np.exp(np.float64(C_SHIFT)) / BETA5
    for c, r in enumerate(res.results):
        blk = np.asarray(r["res"], dtype=np.float64)        # [P, N_ACC+ROWS]
        acc = blk[:, :N_ACC].sum(axis=0)                    # per accum col
        sact = np.empty(ROWS)
        col = 0
        for i, chunks in enumerate(ROW_PLAN):
            sact[i] = acc[col : col + len(chunks)].sum()
            col += len(chunks)
        sd = blk[0, N_ACC : N_ACC + ROWS]                   # [ROWS]
        s_tot = sact + scale_d * sd                         # [ROWS]
        lse = np.log(s_tot) - C_SHIFT
        total += T * lse.sum() - pick_sums[c]
    loss = total / B
    return np.array(np.float32(loss))
